# revision 1
# baseline (speedup 1.0000x reference)
"""Trainium2 Bass kernel for nn_AttentionSortNet (sparse_attention).

Per bh slice (data-parallel over bh across 8 cores):
  b_q = bucket-mean(q), b_k = bucket-mean(k)          (64 buckets x 128 elems)
  sq = b_q + q_pos, sk = b_k + k_pos
  R  = sq @ sk^T                                       (64 x 64)
  E0 = exp((ln(relu(R)+eps) + gumbel) / T)
  8x Sinkhorn in prob domain: E /= rowsum; E /= colsum
  out = E

Device mapping (per core, 4 bh = 2 bh-pairs):
  - q/k pair loads: 4 x 1 MiB HWDGE DMAs per (pair, tensor) into fp32r tiles
    [128, 2048] with partition = (bh-in-pair, bucket), free = (seq r, dim d).
  - bucket means: 4 accumulating fp32r matmuls per chunk with (I/128)
    stationary (PE as strided accumulator), one DVE reduce over the last
    factor 8, trailing each chunk so only ~1 chunk serializes after the DMA.
  - PE HAM warmup + ACT table pre-warm run during the first DMA.
  - R: fp32 matmuls on transposed mean tiles.
  - Sinkhorn: all 4 bh packed in one [128, 128] tile; per half-iteration:
    DVE reduce -> DVE broadcast divide -> PE transpose.
"""
import sys

sys.path.insert(0, "/opt/trn_rl_repo")

import numpy as np

import concourse.bass as bass
import concourse.bacc as bacc
import concourse.mybir as mybir
from concourse import tile
from concourse.bass_utils import run_bass_kernel_spmd

HEADS = 8
BUCKETS = 64
DIM = 64
TEMP = 0.7
EPS = 1e-6
N_CORES = 8
BH = 32
SEQ = 8192
NBH = BH // N_CORES        # 4 bh per core
PAIRS = NBH // 2           # 2 bh-pairs per core
SINKHORN_ITER = 8
CHUNKS = 4                 # 1 MiB DMA chunks per (pair, tensor)
WARMUP_MM = 14

F32 = mybir.dt.float32
F32R = mybir.dt.float32r
AF = mybir.ActivationFunctionType
AX = mybir.AxisListType
ALU = mybir.AluOpType


def _build_program():
    nc = bacc.Bacc("TRN2", target_bir_lowering=False, debug=False, num_devices=N_CORES)

    q_d = nc.dram_tensor("q", [NBH, SEQ, DIM], F32, kind="ExternalInput")
    k_d = nc.dram_tensor("k", [NBH, SEQ, DIM], F32, kind="ExternalInput")
    qp_d = nc.dram_tensor("qpos", [NBH, BUCKETS, DIM], F32, kind="ExternalInput")
    kp_d = nc.dram_tensor("kpos", [NBH, BUCKETS, DIM], F32, kind="ExternalInput")
    g_d = nc.dram_tensor("gumbel", [NBH, BUCKETS, BUCKETS], F32, kind="ExternalInput")
    id_d = nc.dram_tensor("ident", [128, 128], F32, kind="ExternalInput")
    idm_d = nc.dram_tensor("identm", [128, 128], F32, kind="ExternalInput")
    out_d = nc.dram_tensor("out", [NBH, BUCKETS, BUCKETS], F32, kind="ExternalOutput")

    with tile.TileContext(nc) as tc:
        with (
            tc.tile_pool(name="const", bufs=1) as constp,
            tc.tile_pool(name="data", bufs=6) as datap,
            tc.tile_pool(name="work", bufs=2) as workp,
            tc.tile_pool(name="persist", bufs=1) as persistp,
            tc.tile_pool(name="small", bufs=2) as smallp,
            tc.tile_pool(name="epool", bufs=2) as ep,
            tc.tile_pool(name="pacc", bufs=3, space=bass.MemorySpace.PSUM) as pacc,
            tc.tile_pool(name="ptr", bufs=2, space=bass.MemorySpace.PSUM) as ptr,
            tc.tile_pool(name="pR", bufs=1, space=bass.MemorySpace.PSUM) as pR,
            tc.tile_pool(name="pE", bufs=2, space=bass.MemorySpace.PSUM) as pE,
        ):
            # fp32r mean weights first on the SWDGE queue (cast fp32 -> fp32r),
            # before the big q/k loads; plain consts via HWDGE.
            identm = constp.tile([128, 128], F32R, tag="identm")
            nc.gpsimd.dma_start(identm[:], idm_d[:])
            ident = constp.tile([128, 128], F32, tag="ident")
            nc.sync.dma_start(ident[:], id_d[:])

            epst = constp.tile([128, 1], F32, tag="eps")
            nc.vector.memset(epst[:], EPS)

            # ACT table pre-warm: load Relu/Ln/Exp LUTs during the DMA wait
            tw = constp.tile([128, 1], F32, tag="tw")
            nc.scalar.activation(tw[:], epst[:], AF.Relu)
            nc.scalar.activation(tw[:], tw[:], AF.Ln, bias=epst[:])
            nc.scalar.activation(tw[:], tw[:], AF.Exp)

            # pos embeddings / gumbel, pair-stacked: [128, 2, 64] with
            # partitions 0:64 = bh {0, 2} (even in pair), 64:128 = bh {1, 3}.
            def load_stacked(dst, src_handle):
                v = src_handle[:].rearrange("(p v) r d -> v r p d", p=2, v=2)
                nc.sync.dma_start(dst[0:64, :, :], v[0])
                nc.sync.dma_start(dst[64:128, :, :], v[1])

            posq = persistp.tile([128, PAIRS, DIM], F32, tag="posq")
            load_stacked(posq, qp_d)
            posk = persistp.tile([128, PAIRS, DIM], F32, tag="posk")
            load_stacked(posk, kp_d)
            gum = persistp.tile([128, PAIRS, BUCKETS], F32, tag="gum")
            load_stacked(gum, g_d)

            E0 = ep.tile([128, 128], F32, tag="E")
            seed = persistp.tile([128, 2], F32, tag="seed")  # E0 row sums

            for pi in range(PAIRS):
                sT = {}
                for nm, src, pos in (("q", q_d, posq), ("k", k_d, posk)):
                    # [2, 8192, 64] -> [128, 4, 2048]: partition = (bh, bucket),
                    # chunk c, free = (seq-in-bucket r:32, dim d:64), contiguous.
                    view = src[2 * pi : 2 * pi + 2].rearrange(
                        "b (bu c rl) d -> (b bu) c (rl d)",
                        bu=BUCKETS, c=CHUNKS, rl=128 // CHUNKS,
                    )
                    acc = pacc.tile([128, DIM, 8], F32, tag="acc")
                    for c in range(CHUNKS):
                        chunk = datap.tile([128, (128 // CHUNKS) * DIM], F32R, tag="data")
                        nc.gpsimd.dma_start(chunk[:], view[:, c])
                        ro = (128 // CHUNKS) // 8
                        dv = chunk[:].rearrange(
                            "p (ro ri d) -> p ro d ri", ro=ro, ri=8, d=DIM
                        )
                        for j in range(ro):
                            nc.tensor.matmul(
                                acc[:],
                                identm[:],
                                dv[:, j],
                                start=(c == 0 and j == 0),
                                stop=(c == CHUNKS - 1 and j == ro - 1),
                            )

                    # finish mean over ri:8, add pos -> s [128(bh,bu), 64(d)]
                    s_sb = workp.tile([128, DIM], F32, tag="s")
                    nc.vector.reduce_sum(s_sb[:], acc[:], axis=AX.X)
                    nc.vector.tensor_add(s_sb[:], s_sb[:], pos[:, pi, :])

                    # transpose to [64(d), 128(bh,bu)] for the R contraction
                    tps = ptr.tile([64, 128], F32, tag="tp")
                    nc.tensor.transpose(tps[:], s_sb[:], ident[:])
                    t_sb = persistp.tile([64, 128], F32, tag=f"sT{nm}{pi}")
                    nc.vector.tensor_copy(t_sb[:], tps[:])
                    sT[nm] = t_sb

                # R[i, j] = sum_d sq[i, d] sk[j, d]; bh pair stacked on partitions
                Rps = pR.tile([128, BUCKETS], F32, tag="R")
                for v in range(2):
                    nc.tensor.matmul(
                        Rps[64 * v : 64 * (v + 1), :],
                        sT["q"][:, 64 * v : 64 * (v + 1)],
                        sT["k"][:, 64 * v : 64 * (v + 1)],
                        start=True,
                        stop=True,
                    )

                # E0 quadrant-column: exp((ln(relu(R)+eps) + g) / T),
                # with its row sums accumulated as the first Sinkhorn seed.
                t1 = workp.tile([128, BUCKETS], F32, tag="t1a")
                nc.scalar.activation(t1[:], Rps[:], AF.Relu)
                t2 = workp.tile([128, BUCKETS], F32, tag="t1b")
                nc.scalar.activation(t2[:], t1[:], AF.Ln, bias=epst[:])
                nc.vector.tensor_add(t2[:], t2[:], gum[:, pi, :])
                nc.scalar.activation(
                    E0[:, 64 * pi : 64 * (pi + 1)], t2[:], AF.Exp,
                    scale=1.0 / TEMP,
                    accum_out=seed[:, pi : pi + 1],
                )
                if pi == 0:
                    # re-warm the Ln LUT off the critical path so pair 1's
                    # prologue only pays one table switch (the Exp reload).
                    # Reading E0 pins this after pair 0's Exp in the schedule.
                    nc.scalar.activation(tw[:], E0[:, 0:1], AF.Ln, bias=epst[:])

            # Sinkhorn, prob domain. E packed [[E0, E2], [E1, E3]]:
            # partition = rows of bh (v), free half h selects bh = 2h + v.
            cur = E0
            for step in range(2 * SINKHORN_ITER):
                if step == 0:
                    ssum = seed
                else:
                    ssum = smallp.tile([128, 2], F32, tag="ss")
                    nc.vector.reduce_sum(
                        ssum[:], cur[:].rearrange("p (h j) -> p h j", h=2), axis=AX.X
                    )
                rs = smallp.tile([128, 2], F32, tag="rs")
                nc.vector.reciprocal(rs[:], ssum[:])
                nxt = ep.tile([128, 128], F32, tag="E")
                nc.vector.tensor_tensor(
                    out=nxt[:].rearrange("p (h j) -> p h j", h=2),
                    in0=cur[:].rearrange("p (h j) -> p h j", h=2),
                    in1=rs[:].unsqueeze(-1).broadcast_to((128, 2, BUCKETS)),
                    op=ALU.mult,
                )
                tp = pE.tile([128, 128], F32, tag="et")
                nc.tensor.transpose(tp[:], nxt[:], ident[:])
                cur = tp

            out_sb = persistp.tile([128, 128], F32, tag="osb")
            nc.vector.tensor_copy(out_sb[:], cur[:])
            for h in range(2):
                eng = nc.sync if h == 0 else nc.scalar
                for v in range(2):
                    eng.dma_start(
                        out_d[2 * h + v],
                        out_sb[64 * v : 64 * (v + 1), 64 * h : 64 * (h + 1)],
                    )

    nc.compile()
    return nc


_NC = None


def _get_program():
    global _NC
    if _NC is None:
        _NC = _build_program()
    return _NC


def _make_in_maps(inputs):
    q = np.ascontiguousarray(inputs["q"], dtype=np.float32)
    k = np.ascontiguousarray(inputs["k"], dtype=np.float32)
    qpe = np.asarray(inputs["q_pos_emb"], dtype=np.float32)
    kpe = np.asarray(inputs["k_pos_emb"], dtype=np.float32)
    g = np.ascontiguousarray(inputs["gumbel"], dtype=np.float32)

    b = BH // HEADS
    qpos = np.broadcast_to(qpe, (b, HEADS, BUCKETS, DIM)).reshape(BH, BUCKETS, DIM)
    kpos = np.broadcast_to(kpe, (b, HEADS, BUCKETS, DIM)).reshape(BH, BUCKETS, DIM)
    ident = np.eye(128, dtype=np.float32)
    identm = (np.eye(128) / 128.0).astype(np.float32)

    in_maps = []
    for c in range(N_CORES):
        sl = slice(NBH * c, NBH * (c + 1))
        in_maps.append(
            {
                "q": np.ascontiguousarray(q[sl]),
                "k": np.ascontiguousarray(k[sl]),
                "qpos": np.ascontiguousarray(qpos[sl]),
                "kpos": np.ascontiguousarray(kpos[sl]),
                "gumbel": np.ascontiguousarray(g[sl]),
                "ident": ident,
                "identm": identm,
            }
        )
    return in_maps


def run(inputs, trace=False):
    nc = _get_program()
    in_maps = _make_in_maps(inputs)
    res = run_bass_kernel_spmd(
        nc, in_maps, core_ids=list(range(N_CORES)), trace=trace
    )
    out = np.concatenate(
        [res.results[c]["out"] for c in range(N_CORES)], axis=0
    ).astype(np.float32)
    return out, res


def kernel(**inputs) -> np.ndarray:
    out, _ = run(inputs, trace=False)
    return out

